# revision 1
# baseline (speedup 1.0000x reference)
"""DeepseekV2-Lite MoE (group GEMM) on 8 TRN2 NeuronCores.

Strategy (expert-parallel, host-routed):
  - Host (numpy, fp32): gate logits -> softmax -> top-6, per-expert token
    lists (stable sort order, capacity-clamped), expert->core assignment
    (8 experts per core), gather of token activations into per-core
    dispatch buffers, bf16 casts and layout transforms.
  - Device (per core, Bass/Tile, bf16 matmuls with fp32 PSUM accum):
      * routed experts: for each of its 8 experts, a token-stationary
        SwiGLU MLP over the (padded) token segment:
          g,u accumulated over H k-tiles, silu(g)*u -> PE-transpose ->
          down projection, scaled by the top-k gate weight.
      * shared experts: an F_SH/8 slice of the dense SwiGLU MLP over all
        tokens -> partial [T, H] output.
  - Host: scatter-add routed slot outputs back to tokens (weights already
    applied on device), sum shared partials across cores.
"""

import os
from contextlib import ExitStack
from dataclasses import dataclass, field

import ml_dtypes
import numpy as np

H = 2048
F = 1408
E = 64
TOP_K = 6
CAP = 512
SCALE = 1.0
F_SH = 2 * F
B, S = 1, 2048
T = B * S
N_CORES = 8
P = 128

BF16 = ml_dtypes.bfloat16

LAST_EXEC_NS = None
_LAST_RUN = None
_LAST_CFG = None


@dataclass
class MoECfg:
    H: int = H                  # hidden size
    T: int = T                  # tokens
    F: int = F                  # routed intermediate
    fchunks: tuple = ((0, 256), (256, 256), (512, 256), (768, 256),
                      (1024, 256), (1280, 128))  # (start, width) of F
    NCAP: int = 256             # per-expert slot capacity (multiple of 128)
    E_LOC: int = 8              # experts per core
    FSH: int = 352              # shared intermediate slice per core (real)
    FSH_PAD: int = 384          # padded to multiple of 128, <= 512
    NCHUNK: int = 1024          # down-proj dma chunk width (matmuls of <=512)
    dma_split: int = 1          # split big weight DMAs into this many pieces
    act_silu: bool = True       # False: Sigmoid+mul (CoreSim-compatible)

    @property
    def KT(self):
        return self.H // P

    @property
    def KF(self):
        return self.F // P

    @property
    def NSLOT(self):
        return self.E_LOC * self.NCAP

    @property
    def MT(self):
        return self.NCAP // P

    @property
    def NQ(self):
        return self.H // self.NCHUNK

    @property
    def KSH(self):
        return self.FSH_PAD // P


def build_moe_program(cfg: MoECfg, repeat: int = 1):
    """Build the per-core Bass program. repeat>1 wraps the body in a device
    loop (benchmarking: slope over repeat cancels host dispatch overhead)."""
    import contextlib
    import concourse.bass as bass  # noqa: F401
    import concourse.mybir as mybir
    from concourse import bacc
    import concourse.tile as tile
    from concourse.masks import make_identity

    bf = mybir.dt.bfloat16
    f32 = mybir.dt.float32
    SILU = (mybir.ActivationFunctionType.Silu if cfg.act_silu
            else mybir.ActivationFunctionType.Sigmoid)

    KT, KF, NSLOT, MT, NQ, KSH = cfg.KT, cfg.KF, cfg.NSLOT, cfg.MT, cfg.NQ, cfg.KSH
    NCH = cfg.NCHUNK
    TMT = cfg.T // P  # token m-tiles (shared phase)

    nc = bacc.Bacc("TRN2", target_bir_lowering=False, debug=False)

    # ---- DRAM parameters -------------------------------------------------
    xdT = nc.dram_tensor("xdT", [KT, P, NSLOT], bf, kind="ExternalInput").ap()
    xT = nc.dram_tensor("xT", [KT, P, cfg.T], bf, kind="ExternalInput").ap()
    wgu = [
        nc.dram_tensor(f"wgu{i}", [cfg.E_LOC, KT, P, 2 * fw], bf, kind="ExternalInput").ap()
        for i, (st, fw) in enumerate(cfg.fchunks)
    ]
    wd = nc.dram_tensor("wd", [cfg.E_LOC, NQ, KF, P, NCH], bf, kind="ExternalInput").ap()
    wslot = nc.dram_tensor("wslot", [P, NSLOT // P], f32, kind="ExternalInput").ap()
    shgu = nc.dram_tensor("shgu", [KT, P, 2 * cfg.FSH_PAD], bf, kind="ExternalInput").ap()
    shd = nc.dram_tensor("shd", [KSH, P, cfg.H], bf, kind="ExternalInput").ap()

    d_out = nc.dram_tensor("d_out", [NSLOT, cfg.H], bf, kind="ExternalOutput").ap()
    ysh_out = nc.dram_tensor("ysh", [cfg.T, cfg.H], bf, kind="ExternalOutput").ap()

    d_out_t = d_out.rearrange("(mt p) h -> mt p h", p=P)
    ysh_out_t = ysh_out.rearrange("(mt p) h -> mt p h", p=P)

    with tile.TileContext(nc) as tc:
        with ExitStack() as top:
            const_pool = top.enter_context(tc.tile_pool(name="const", bufs=1))
            ident = const_pool.tile([P, P], bf)
            make_identity(nc, ident)
            wslot_sb = const_pool.tile([P, NSLOT // P], f32)
            nc.sync.dma_start(wslot_sb, wslot)

            rep_ctx = tc.For_i(0, repeat, 1) if repeat > 1 else contextlib.nullcontext()
            with rep_ctx:
              # ================= routed experts =================
              with ExitStack() as ph:
                  xd_pool = ph.enter_context(tc.tile_pool(name="xd", bufs=1))
                  wgu_pool = ph.enter_context(tc.tile_pool(name="wgu", bufs=6))
                  wd_pool = ph.enter_context(tc.tile_pool(name="wdp", bufs=2))
                  yt_pool = ph.enter_context(tc.tile_pool(name="yt", bufs=2))
                  tmp_pool = ph.enter_context(tc.tile_pool(name="tmp", bufs=2))
                  out_pool = ph.enter_context(tc.tile_pool(name="outp", bufs=3))
                  psgu_pool = ph.enter_context(tc.tile_pool(name="psgu", bufs=2, space="PSUM"))
                  pst_pool = ph.enter_context(tc.tile_pool(name="pst", bufs=2, space="PSUM"))
                  psd_pool = ph.enter_context(tc.tile_pool(name="psd", bufs=4, space="PSUM"))

                  # split lhsT source into k-groups: finer DMA/dep granularity
                  NG = 4 if KT % 4 == 0 else 1
                  KG = KT // NG
                  xdT_sbs = []
                  for g in range(NG):
                      xg = xd_pool.tile([P, KG, NSLOT], bf, tag=f"xdT{g}", name=f"xdT{g}")
                      nc.gpsimd.dma_start(xg, xdT[g * KG : (g + 1) * KG].rearrange("k p s -> p k s"))
                      xdT_sbs.append(xg)

                  NH = 2 if KT % 2 == 0 else 1
                  KH = KT // NH

                  WGU_W = max(2 * fw for (_s, fw) in cfg.fchunks)
                  for e in range(cfg.E_LOC):
                      yT = yt_pool.tile([P, KF, cfg.NCAP], bf, tag="yT")
                      # ---- gate/up + swiglu + transpose, per F-chunk ----
                      for ci, (fstart, fw) in enumerate(cfg.fchunks):
                          wts = []
                          for h in range(NH):
                              wt = wgu_pool.tile([P, KH, WGU_W], bf, tag="wgu",
                                                 name=f"wgu_e{e}c{ci}h{h}")
                              nc.sync.dma_start(
                                  wt[:, :, : 2 * fw],
                                  wgu[ci][e, h * KH : (h + 1) * KH].rearrange("k p f -> p k f"),
                              )
                              wts.append(wt)
                          fused = 2 * fw <= 512
                          for m in range(MT):
                              if fused:
                                  pgu = psgu_pool.tile([P, 512], f32, tag="pgu")
                                  uof = fw
                              else:
                                  pgu = psgu_pool.tile([P, 1024], f32, tag="pgu")
                                  uof = 512
                              for k in range(KT):
                                  lhs = xdT_sbs[k // KG][:, k % KG, e * cfg.NCAP + m * P : e * cfg.NCAP + (m + 1) * P]
                                  wtk = wts[k // KH][:, k % KH]
                                  if fused:
                                      nc.tensor.matmul(
                                          pgu[:, : 2 * fw], lhs, wtk[:, : 2 * fw],
                                          start=(k == 0), stop=(k == KT - 1),
                                      )
                                  else:
                                      nc.tensor.matmul(
                                          pgu[:, :fw], lhs, wtk[:, :fw],
                                          start=(k == 0), stop=(k == KT - 1),
                                      )
                                      nc.tensor.matmul(
                                          pgu[:, uof : uof + fw], lhs, wtk[:, fw : 2 * fw],
                                          start=(k == 0), stop=(k == KT - 1),
                                      )
                              sil = tmp_pool.tile([P, 512], f32, tag="sil")
                              nc.scalar.activation(sil[:, :fw], pgu[:, :fw], SILU)
                              if not cfg.act_silu:
                                  nc.vector.tensor_mul(sil[:, :fw], sil[:, :fw], pgu[:, :fw])
                              yck = tmp_pool.tile([P, 512], bf, tag="yck")
                              nc.vector.tensor_mul(yck[:, :fw], sil[:, :fw], pgu[:, uof : uof + fw])
                              for ft in range(fw // P):
                                  pt = pst_pool.tile([P, P], bf, tag="pt")
                                  nc.tensor.transpose(pt, yck[:, ft * P : (ft + 1) * P], ident)
                                  nc.vector.tensor_copy(
                                      yT[:, fstart // P + ft, m * P : (m + 1) * P], pt
                                  )
                      # ---- down projection ----
                      NCW = min(512, NCH)
                      NN = NCH // NCW
                      outs = [out_pool.tile([P, cfg.H], bf, tag="dout", name=f"dout{m}") for m in range(MT)]
                      for q in range(NQ):
                          wdt = wd_pool.tile([P, KF, NCH], bf, tag="wd")
                          nc.scalar.dma_start(wdt, wd[e, q].rearrange("kf p n -> p kf n"))
                          pds = [psd_pool.tile([P, NCW], f32, tag="pd", name=f"pd{m}_{nn}")
                                 for m in range(MT) for nn in range(NN)]
                          for kf in range(KF):
                              for m in range(MT):
                                  for nn in range(NN):
                                      nc.tensor.matmul(
                                          pds[m * NN + nn], yT[:, kf, m * P : (m + 1) * P],
                                          wdt[:, kf, nn * NCW : (nn + 1) * NCW],
                                          start=(kf == 0), stop=(kf == KF - 1),
                                      )
                          for m in range(MT):
                              for nn in range(NN):
                                  nc.vector.tensor_scalar_mul(
                                      outs[m][:, q * NCH + nn * NCW : q * NCH + (nn + 1) * NCW],
                                      pds[m * NN + nn],
                                      wslot_sb[:, e * MT + m : e * MT + m + 1],
                                  )
                      for m in range(MT):
                          nc.gpsimd.dma_start(d_out_t[e * MT + m], outs[m])

              # ================= shared experts (F_SH slice) =================
              with ExitStack() as ph:
                  x_pool = ph.enter_context(tc.tile_pool(name="xfull", bufs=1))
                  shw_pool = ph.enter_context(tc.tile_pool(name="shw", bufs=1))
                  ysht_pool = ph.enter_context(tc.tile_pool(name="ysht", bufs=2))
                  tmp_pool = ph.enter_context(tc.tile_pool(name="tmps", bufs=2))
                  out_pool = ph.enter_context(tc.tile_pool(name="outs", bufs=2))
                  psgu_pool = ph.enter_context(tc.tile_pool(name="psgus", bufs=2, space="PSUM"))
                  pst_pool = ph.enter_context(tc.tile_pool(name="psts", bufs=2, space="PSUM"))
                  psd_pool = ph.enter_context(tc.tile_pool(name="psds", bufs=2, space="PSUM"))

                  NG = 4 if KT % 4 == 0 else 1
                  KG = KT // NG
                  xT_sbs = []
                  for g in range(NG):
                      xg = x_pool.tile([P, KG, cfg.T], bf, tag=f"xT{g}", name=f"xT{g}")
                      nc.gpsimd.dma_start(xg, xT[g * KG : (g + 1) * KG].rearrange("k p s -> p k s"))
                      xT_sbs.append(xg)
                  shgu_sbs = []
                  for g in range(NG):
                      sg = shw_pool.tile([P, KG, 2 * cfg.FSH_PAD], bf, tag=f"shgu{g}", name=f"shgu{g}")
                      nc.scalar.dma_start(sg, shgu[g * KG : (g + 1) * KG].rearrange("k p f -> p k f"))
                      shgu_sbs.append(sg)
                  shd_sb = shw_pool.tile([P, KSH, cfg.H], bf)
                  nc.scalar.dma_start(shd_sb, shd.rearrange("kf p n -> p kf n"))

                  FP = cfg.FSH_PAD
                  for mt in range(TMT):
                      pgu = psgu_pool.tile([P, 1024], f32, tag="pgus")
                      for k in range(KT):
                          lhs = xT_sbs[k // KG][:, k % KG, mt * P : (mt + 1) * P]
                          sgk = shgu_sbs[k // KG][:, k % KG]
                          nc.tensor.matmul(
                              pgu[:, :FP], lhs, sgk[:, :FP],
                              start=(k == 0), stop=(k == KT - 1),
                          )
                          nc.tensor.matmul(
                              pgu[:, 512 : 512 + FP], lhs, sgk[:, FP : 2 * FP],
                              start=(k == 0), stop=(k == KT - 1),
                          )
                      sil = tmp_pool.tile([P, FP], f32, tag="sils")
                      nc.scalar.activation(sil, pgu[:, :FP], SILU)
                      if not cfg.act_silu:
                          nc.vector.tensor_mul(sil, sil, pgu[:, :FP])
                      ysh = tmp_pool.tile([P, FP], bf, tag="yshs")
                      nc.vector.tensor_mul(ysh, sil, pgu[:, 512 : 512 + FP])
                      yshT = ysht_pool.tile([P, KSH, P], bf, tag="yshT")
                      for ft in range(KSH):
                          pt = pst_pool.tile([P, P], bf, tag="pts")
                          nc.tensor.transpose(pt, ysh[:, ft * P : (ft + 1) * P], ident)
                          nc.vector.tensor_copy(yshT[:, ft, :], pt)
                      osh = out_pool.tile([P, cfg.H], bf, tag="osh")
                      SNC = min(512, cfg.H)
                      SNQ = cfg.H // SNC
                      for half in range(max(1, SNQ // 2)):
                          nns = min(2, SNQ)
                          pds = [psd_pool.tile([P, SNC], f32, tag="pds", name=f"pds{nn}") for nn in range(nns)]
                          for kf in range(KSH):
                              for nn in range(nns):
                                  nc.tensor.matmul(
                                      pds[nn],
                                      yshT[:, kf, :],
                                      shd_sb[:, kf, (half * nns + nn) * SNC : (half * nns + nn + 1) * SNC],
                                      start=(kf == 0), stop=(kf == KSH - 1),
                                  )
                          for nn in range(nns):
                              nc.scalar.copy(
                                  osh[:, (half * nns + nn) * SNC : (half * nns + nn + 1) * SNC],
                                  pds[nn],
                              )
                      nc.gpsimd.dma_start(ysh_out_t[mt], osh)

    nc.compile()
    return nc


# ---------------------------------------------------------------------------
# Host-side routing / sharding / combine
# ---------------------------------------------------------------------------

def _route(x32, gate_w):
    """fp32 gate: softmax + top-k. Returns (topk_idx [T,K] int32, topk_w [T,K] f32)."""
    logits = x32 @ gate_w.T.astype(np.float32)
    logits -= logits.max(-1, keepdims=True)
    np.exp(logits, out=logits)
    logits /= logits.sum(-1, keepdims=True)
    idx = np.argpartition(-logits, TOP_K - 1, axis=-1)[:, :TOP_K]
    w = np.take_along_axis(logits, idx, -1) * SCALE
    return idx.astype(np.int64), w.astype(np.float32)


def kernel(hidden_states, gate_w, w_gate, w_up, w_down, sh_gate, sh_up, sh_down):
    global LAST_EXEC_NS
    from concourse.bass_utils import run_bass_kernel_spmd

    x32 = np.ascontiguousarray(hidden_states, dtype=np.float32).reshape(T, H)

    # ---- host routing ----
    topk_idx, topk_w = _route(x32, np.asarray(gate_w, np.float32))
    eid = topk_idx.reshape(-1)                      # [T*K]
    order = np.argsort(eid, kind="stable")          # slots sorted by expert
    counts = np.bincount(eid, minlength=E)
    starts = np.concatenate([[0], np.cumsum(counts)[:-1]])
    wflat = topk_w.reshape(-1)

    ncap = 256
    maxc = int(min(counts.max(), CAP))
    while ncap < maxc:
        ncap += 128
    cfg = MoECfg(NCAP=ncap)

    # ---- per-core dispatch metadata ----
    # core c owns experts [8c, 8c+8); expert j on core c has slot rows
    # [j*NCAP, j*NCAP + c_e) in that core's dispatch buffer.
    x16T = np.ascontiguousarray(x32.T.astype(BF16))           # [H, T] bf16
    xT_arr = x16T.reshape(cfg.KT, P, T)                       # replicated

    # bf16 weight stacks, laid out per core
    wg16 = np.asarray(w_gate, np.float32).astype(BF16)        # [E, H, F]
    wu16 = np.asarray(w_up, np.float32).astype(BF16)
    wdn16 = np.asarray(w_down, np.float32).astype(BF16)       # [E, F, H]
    shg16 = np.asarray(sh_gate, np.float32).astype(BF16)      # [F_SH, H]
    shu16 = np.asarray(sh_up, np.float32).astype(BF16)
    shdn16 = np.asarray(sh_down, np.float32).astype(BF16)     # [H, F_SH]

    in_maps = []
    core_meta = []
    for c in range(N_CORES):
        es = np.arange(c * cfg.E_LOC, (c + 1) * cfg.E_LOC)
        # slot -> token map (pad slots point at token 0 with weight 0)
        tok = np.zeros(cfg.NSLOT, np.int64)
        wsl = np.zeros(cfg.NSLOT, np.float32)
        seg = []
        for j, eg in enumerate(es):
            ce = int(min(counts[eg], CAP))
            rows = order[starts[eg] : starts[eg] + ce]
            tok[j * cfg.NCAP : j * cfg.NCAP + ce] = rows // TOP_K
            wsl[j * cfg.NCAP : j * cfg.NCAP + ce] = wflat[rows]
            seg.append((j, eg, ce, rows))
        core_meta.append(seg)

        xdT_arr = np.ascontiguousarray(x16T[:, tok]).reshape(cfg.KT, P, cfg.NSLOT)

        fch = cfg.fchunks
        im = {
            "xdT": xdT_arr,
            "xT": xT_arr,
            "wd": np.ascontiguousarray(
                wdn16[es]
                .reshape(cfg.E_LOC, cfg.KF, P, cfg.NQ, cfg.NCHUNK)
                .transpose(0, 3, 1, 2, 4)
            ),
            "wslot": np.ascontiguousarray(wsl.reshape(cfg.NSLOT // P, P).T),
        }
        for i, (st, fw) in enumerate(fch):
            gpart = wg16[es, :, st : st + fw].reshape(cfg.E_LOC, cfg.KT, P, fw)
            upart = wu16[es, :, st : st + fw].reshape(cfg.E_LOC, cfg.KT, P, fw)
            im[f"wgu{i}"] = np.ascontiguousarray(np.concatenate([gpart, upart], axis=-1))
        # shared slice for this core
        sl = slice(c * cfg.FSH, (c + 1) * cfg.FSH)
        sg = np.zeros((H, cfg.FSH_PAD), BF16)
        su = np.zeros((H, cfg.FSH_PAD), BF16)
        sg[:, : cfg.FSH] = shg16[sl].T
        su[:, : cfg.FSH] = shu16[sl].T
        im["shgu"] = np.ascontiguousarray(
            np.concatenate([sg, su], axis=-1).reshape(cfg.KT, P, 2 * cfg.FSH_PAD)
        )
        sd = np.zeros((cfg.FSH_PAD, H), BF16)
        sd[: cfg.FSH] = shdn16[:, sl].T
        im["shd"] = np.ascontiguousarray(sd.reshape(cfg.KSH, P, H))
        in_maps.append(im)

    # ---- build + run ----
    nc = build_moe_program(cfg)
    trace = os.environ.get("MOE_TRACE", "0") == "1"
    res = run_bass_kernel_spmd(
        nc, in_maps, core_ids=list(range(N_CORES)), trace=trace,
    )
    LAST_EXEC_NS = res.exec_time_ns
    global _LAST_RUN, _LAST_CFG
    _LAST_RUN = (nc, in_maps)
    _LAST_CFG = cfg

    # ---- combine ----
    out_exp = np.zeros((T * TOP_K, H), np.float32)
    y = np.zeros((T, H), np.float32)
    for c in range(N_CORES):
        d = np.asarray(res.results[c]["d_out"], dtype=np.float32)  # weighted slots
        for j, eg, ce, rows in core_meta[c]:
            out_exp[rows] = d[j * cfg.NCAP : j * cfg.NCAP + ce]
        y += np.asarray(res.results[c]["ysh"], dtype=np.float32)
    y += out_exp.reshape(T, TOP_K, H).sum(axis=1)
    return y.reshape(B, S, H).astype(hidden_states.dtype)

